# revision 35
# baseline (speedup 1.0000x reference)
"""CycleRNN (2-layer LSTM with output feedback) Trainium2 kernel.

Strategy: data-parallel over batch (B=256 -> 32 per core, 8 cores, zero
cross-core communication; the time loop is sequential per core).

Per-core design (v2):
  * All weights AND the entire x sequence live in SBUF in bf16 (x is
    77KB/partition), so the steady-state loop issues no input DMAs at all;
    only y goes out via DMA once per unrolled iteration.
  * Matmuls keep the WEIGHTS stationary (lhsT) and stream the batch
    (N=32) as the moving operand; a 128x128 bf16 weight tile streams in
    ~25ns with the weight load fully hidden.
  * Activations are transposed+packed: a [B=32, F] tensor is stored as
    [128 partitions = F mod 128, (F//128)*32 cols] so matmul outputs
    ([gate-tile, batch] in PSUM) feed the next matmul with no transposes.
  * The LSTM cell is restructured for latency: sigmoid(i,f) is ONE
    256-col Act instr gated on the i/f PSUM regions only (the tile
    framework tracks deps at AP-range granularity), tanh(g) follows, and
    sigmoid(o) is deferred past the DVE c_new chain; o-gate matmul tiles
    are emitted last.  y_t copies run on the Vector engine so the Act
    engine only ever runs Relu/Sigmoid/Tanh (no act-table thrash).
  * z_t = relu(x_t @ w1.T) is split: the 12 x-only k-tiles are
    pre-accumulated into PSUM one step early (PE filler during the cell
    windows); only the 4 feedback (y_{t-1}) tiles sit on the critical
    path.
  * The PE instruction stream is software-pipelined: the w_hh matmuls of
    layer l (which only need state from the previous step) fill the PE
    while the other engines run the LSTM cell math.
"""

import os
import sys
from contextlib import ExitStack

os.environ.setdefault("MYCRO_LOCAL_CACHE", "1")
sys.path.insert(0, "/opt/trn_rl_repo")

import numpy as np
import ml_dtypes

import concourse.bass as bass
import concourse.mybir as mybir
import concourse.tile as tile
from concourse.bass import ds
from concourse.bass_utils import run_bass_kernel_spmd

BF16 = ml_dtypes.bfloat16

T, B, IN, H, OUT, L = 300, 256, 512, 512, 128, 2
NCORES = 8
BC = B // NCORES          # 32 batch rows per core
G = 4 * H                 # 2048 gates per layer
KT = IN // 128            # 4 k-tiles per 512-feature dim
GM = G // 128             # 16 gate m-tiles
U = int(os.environ.get("KUNROLL", "4"))  # steps unrolled per For_i iteration
STAGGERED = os.environ.get("KSTAG", "0") == "1"
HINTS = os.environ.get("KHINT", "1") == "1"

f32 = mybir.dt.float32
bf16 = mybir.dt.bfloat16
AF = mybir.ActivationFunctionType


def build_program(zero_bias: bool, n_steps: int = T, reps: int = 1):
    nc = bass.Bass()

    # ---------------- DRAM parameters (host-packed layouts) ----------------
    # xT is padded by U zero steps on the host: the mid-body prefetch of the
    # final iteration reads one half-buffer past the real data.
    xT_d = nc.declare_dram_parameter("xT", [n_steps + U, IN, BC], bf16,
                                     isOutput=False)
    w1_d = nc.declare_dram_parameter("w1T", [IN, H], bf16, isOutput=False)
    # W_fb = w1[:, IN-OUT:] @ w2: the y->z feedback collapsed into one
    # matrix, so z_t gets W_fb @ h1_{t-1} directly and the y computation
    # drops off the recurrent critical path.
    wfb_d = nc.declare_dram_parameter("wfbT", [H, H], bf16, isOutput=False)
    wih_d = nc.declare_dram_parameter("wihT", [L, H, G], bf16, isOutput=False)
    whh_d = nc.declare_dram_parameter("whhT", [L, H, G], bf16, isOutput=False)
    w2_d = nc.declare_dram_parameter("w2T", [H, OUT], bf16, isOutput=False)
    b1_d = nc.declare_dram_parameter("b1p", [128, KT], f32, isOutput=False)
    bs_d = nc.declare_dram_parameter("bsp", [L, 128, GM], f32, isOutput=False)
    b2_d = nc.declare_dram_parameter("b2p", [128, 1], f32, isOutput=False)
    ys_d = nc.declare_dram_parameter("ysT", [n_steps, OUT, BC], f32, isOutput=True)
    ys_flat = ys_d[:].rearrange("t o b -> (t o) b")
    xT_flat = xT_d[:].rearrange("t p b -> (t p) b")

    assert U % 2 == 0 and n_steps % U == 0
    UH = U // 2                        # steps per x half-buffer
    XH = UH * KT * BC                  # cols per x half-buffer

    est = ExitStack()
    with est:
        # ---------------- persistent SBUF tensors ----------------
        # x double-buffer: x_a holds steps 0..UH-1 of the iteration, x_b the
        # rest.  x_a for iteration j+1 is prefetched mid-body of iteration j
        # (the cross-iteration z-xpart pre-emission reads it), x_b for
        # iteration j at the top of body j.
        x_ab = [est.enter_context(nc.sbuf_tensor(f"x_{h}", [128, XH], bf16))
                for h in range(2)]
        w1_sb = est.enter_context(nc.sbuf_tensor([128, KT * H], bf16))
        wfb_sb = est.enter_context(nc.sbuf_tensor([128, KT * H], bf16))
        wih_sb = est.enter_context(nc.sbuf_tensor([128, L * KT * G], bf16))
        whh_sb = est.enter_context(nc.sbuf_tensor([128, L * KT * G], bf16))
        w2_sb = est.enter_context(nc.sbuf_tensor([128, KT * OUT], bf16))
        b1_sb = est.enter_context(nc.sbuf_tensor([128, KT], f32))
        bs_sb = est.enter_context(nc.sbuf_tensor([128, L * GM], f32))
        b2_sb = est.enter_context(nc.sbuf_tensor([128, 1], f32))
        x0fb_sb = est.enter_context(nc.sbuf_tensor([128, BC], bf16))
        # state ping-pong: index = step % 2
        h_sb = [[est.enter_context(nc.sbuf_tensor(f"h{l}_{s}", [128, 128], bf16))
                 for s in range(2)] for l in range(L)]
        c_sb = [[est.enter_context(nc.sbuf_tensor(f"c{l}_{s}", [128, 128], f32))
                 for s in range(2)] for l in range(L)]
        # PSUM: 8 banks = 8 single-buffered tensors, one per accumulation
        # REGION.  Tile's hazard tracking is tensor-granular, so giving each
        # gate group (i+f / g / o) its own bank lets the cell activations
        # start as soon as THEIR region finishes accumulating instead of
        # waiting for the layer's full 64-matmul gate set.  Single buffering
        # is safe: every write-after-read pair is already ordered by the
        # step's dataflow (e.g. next step's w_hh fill only runs once h_new
        # exists, which requires this step's gate reads to have finished).
        zp_t = est.enter_context(nc.psum_tensor("zp", [128, 128], f32))
        yp_t = est.enter_context(nc.psum_tensor("yp", [128, BC], f32))
        gif = [est.enter_context(nc.psum_tensor(f"g{l}if", [128, 256], f32))
               for l in range(L)]
        gg = [est.enter_context(nc.psum_tensor(f"g{l}g", [128, 128], f32))
              for l in range(L)]
        go = [est.enter_context(nc.psum_tensor(f"g{l}o", [128, 128], f32))
              for l in range(L)]

        with tile.TileContext(nc) as tc, \
                tc.tile_pool(name="work", bufs=2) as sb_pool:

            # ---------------- one-time loads (one DMA per tensor) ----------
            def emit_xdma(dst, rowbase):
                """Load UH steps of x (transposed bf16) into a half-buffer."""
                nc.sync.dma_start(
                    out=dst[:].rearrange("p (q b) -> p q b", b=BC),
                    in_=xT_flat[ds(rowbase, UH * IN), :].rearrange(
                        "(q p) b -> p q b", p=128),
                )

            emit_xdma(x_ab[0], 0)      # iteration 0, first half
            nc.sync.dma_start(
                out=w1_sb[:].rearrange("p (k h) -> p k h", k=KT),
                in_=w1_d[:].rearrange("(k p) h -> p k h", p=128),
            )
            nc.sync.dma_start(
                out=wfb_sb[:].rearrange("p (k h) -> p k h", k=KT),
                in_=wfb_d[:].rearrange("(k p) h -> p k h", p=128),
            )
            nc.sync.dma_start(
                out=w2_sb[:].rearrange("p (k o) -> p k o", k=KT),
                in_=w2_d[:].rearrange("(k p) o -> p k o", p=128),
            )
            nc.sync.dma_start(
                out=wih_sb[:].rearrange("p (q g) -> p q g", g=G),
                in_=wih_d[:].rearrange("l h g -> (l h) g").rearrange(
                    "(q p) g -> p q g", p=128),
            )
            nc.sync.dma_start(
                out=whh_sb[:].rearrange("p (q g) -> p q g", g=G),
                in_=whh_d[:].rearrange("l h g -> (l h) g").rearrange(
                    "(q p) g -> p q g", p=128),
            )
            nc.sync.dma_start(out=b1_sb[:], in_=b1_d[:])
            nc.sync.dma_start(
                out=bs_sb[:].rearrange("p (l m) -> p l m", l=L),
                in_=bs_d[:].rearrange("l p m -> p l m"),
            )
            nc.sync.dma_start(out=b2_sb[:], in_=b2_d[:])
            # t=0 feedback is ground truth x[0,:, -OUT:] (prologue-only)
            nc.sync.dma_start(out=x0fb_sb[:], in_=xT_d[0, IN - OUT:IN, :])
            # Make SP observe every preamble DMA queue semaphore now, so the
            # loop back-edge drain's wait list only holds loop-body procs
            # (the ISA caps sync-wait commands per instruction).
            for tns in (x_ab[0], w1_sb, wfb_sb, w2_sb, wih_sb, whh_sb, b1_sb,
                        bs_sb, b2_sb, x0fb_sb):
                n = 2 if tns.dtype == bf16 else 1
                nc.sync.value_load(tns[0:1, 0:n].bitcast(mybir.dt.int32))
            # initial h/c = 0 (slot 1 is read by step 0)
            for l in range(L):
                nc.vector.memset(h_sb[l][1][:], 0.0)
                nc.vector.memset(c_sb[l][1][:], 0.0)

            # w slice helpers: stationary lhsT tiles
            def w1_t(k, m):
                return w1_sb[:, k * H + m * 128: k * H + (m + 1) * 128]

            def wfb_t(k, m):
                return wfb_sb[:, k * H + m * 128: k * H + (m + 1) * 128]

            def wih_t(l, k, m):
                o = (l * KT + k) * G
                return wih_sb[:, o + m * 128: o + (m + 1) * 128]

            def whh_t(l, k, m):
                o = (l * KT + k) * G
                return whh_sb[:, o + m * 128: o + (m + 1) * 128]

            def w2_t(k):
                return w2_sb[:, k * OUT:(k + 1) * OUT]

            # ---------------- per-step emitters ----------------
            # Gate m-tile -> (region tensor, col offset, region's first m).
            # PyTorch gate order i,f,g,o in 128-unit blocks: m 0..7 = i,f
            # ("if" region), 8..11 = g, 12..15 = o.
            def g_region(l, m):
                if m < 8:
                    return gif[l], m * 32, 0
                if m < 12:
                    return gg[l], (m - 8) * 32, 8
                return go[l], (m - 12) * 32, 12

            # start=True clears the has_written bits of the WHOLE psum bank,
            # so each region-bank epoch gets exactly one start (its first
            # w_hh MM) and one stop (its last w_ih MM).
            def emit_gh(l, src_h, m0, m1):
                """h-part of layer-l gates for m-tiles [m0, m1)."""
                for m in range(m0, m1):
                    dst, col, mf = g_region(l, m)
                    for k in range(KT):
                        nc.tensor.matmul(
                            dst[:, col:col + 32],
                            whh_t(l, k, m),
                            src_h[:, k * 32:(k + 1) * 32],
                            start=(m == mf and k == 0),
                            stop=False,
                        )

            def emit_gin(l, src):
                """input-part of layer-l gates (accumulates onto h-part).
                Emitted region-major (if, g, o) so the cell activations'
                inputs complete in consumption order; each region's last MM
                closes that bank's epoch."""
                for m in range(GM):
                    dst, col, mf = g_region(l, m)
                    last_m = {0: 7, 8: 11, 12: 15}[mf]
                    for k in range(KT):
                        nc.tensor.matmul(
                            dst[:, col:col + 32],
                            wih_t(l, k, m),
                            src[:, k * 32:(k + 1) * 32],
                            start=False,
                            stop=(m == last_m and k == KT - 1),
                        )

            def x_step(s):
                """Static x AP base for body-step s: (half-buffer, col)."""
                return x_ab[s // UH], (s % UH) * KT * BC

            def emit_z_xpart(s, z_dst):
                """x-only part of z for body-step s%U: k=0..2 (12 MMs).  Runs
                a step ahead of its consumer, as PE filler.  The epoch is
                closed by the step's emit_z_fb (the very last phantom epoch
                never closes, which is harmless -- no sim in this path)."""
                xt, xcol = x_step(s % U)
                for m in range(KT):
                    for k in range(KT - 1):
                        nc.tensor.matmul(
                            z_dst[:, m * 32:(m + 1) * 32],
                            w1_t(k, m),
                            xt[:, xcol + k * BC: xcol + (k + 1) * BC],
                            start=(m == 0 and k == 0),
                            stop=False,
                        )

            def emit_z_hfb(src_h, z_dst):
                """feedback part of z: W_fb @ h1_prev (16 MMs, closes the
                step's z epoch).  For the very first step h1_prev is the
                zero-initialized state, so this contributes nothing and the
                prologue's ground-truth term stands."""
                for m in range(KT):
                    for k in range(KT):
                        nc.tensor.matmul(
                            z_dst[:, m * 32:(m + 1) * 32],
                            wfb_t(k, m),
                            src_h[:, k * 32:(k + 1) * 32],
                            start=False,
                            stop=(m == KT - 1 and k == KT - 1),
                        )

            def emit_zact(z_src):
                """relu on the VECTOR engine (max with 0), so the Act engine
                only ever runs Sigmoid/Tanh and its table cache never
                thrashes."""
                z_bf = sb_pool.tile([128, 128], bf16, tag="z_bf")
                if zero_bias:
                    nc.vector.tensor_scalar_max(z_bf[:], z_src[:, 0:128], 0.0)
                else:
                    for m in range(KT):
                        nc.vector.tensor_scalar(
                            z_bf[:, m * 32:(m + 1) * 32],
                            z_src[:, m * 32:(m + 1) * 32],
                            b1_sb[:, m:m + 1],
                            0.0,
                            mybir.AluOpType.add,
                            mybir.AluOpType.max,
                        )
                return z_bf

            def emit_cell(l, s):
                """gates -> (h_new bf16, c_new f32) into slot s.

                Fast path (zero bias): 3 gate activations -- sigmoid(i,f) as
                one 256-col instr gated only on the "if" bank, tanh(g) on the
                g bank, and sigmoid(o) emitted after the DVE chain kicks off
                so it overlaps c_new."""
                c_old = c_sb[l][1 - s]
                c_new = c_sb[l][s]
                h_new = h_sb[l][s]
                if zero_bias:
                    sif = sb_pool.tile([128, 256], f32, tag="sif")
                    tg = sb_pool.tile([128, 128], f32, tag="tg")
                    s_o = sb_pool.tile([128, 128], f32, tag="s_o")
                    tcn = sb_pool.tile([128, 128], f32, tag="tcn")
                    t1 = sb_pool.tile([128, 128], f32, tag="t1")
                    t2 = sb_pool.tile([128, 128], f32, tag="t2")
                    nc.scalar.activation(sif[:], gif[l][:], AF.Sigmoid)
                    nc.scalar.activation(tg[:], gg[l][:], AF.Tanh)
                    nc.vector.tensor_mul(t1[:], sif[:, 128:256], c_old[:])
                    nc.vector.tensor_mul(t2[:], sif[:, 0:128], tg[:])
                    nc.vector.tensor_add(c_new[:], t1[:], t2[:])
                    nc.scalar.activation(s_o[:], go[l][:], AF.Sigmoid)
                    nc.scalar.activation(tcn[:], c_new[:], AF.Tanh)
                    nc.vector.tensor_mul(h_new[:], s_o[:], tcn[:])
                    return h_new
                s_i = sb_pool.tile([128, 128], f32, tag="s_i")
                s_f = sb_pool.tile([128, 128], f32, tag="s_f")
                tg = sb_pool.tile([128, 128], f32, tag="tg")
                s_o = sb_pool.tile([128, 128], f32, tag="s_o")
                outs = [s_i, s_i, s_i, s_i, s_f, s_f, s_f, s_f,
                        tg, tg, tg, tg, s_o, s_o, s_o, s_o]
                funcs = [AF.Sigmoid] * 8 + [AF.Tanh] * 4 + [AF.Sigmoid] * 4
                for m in range(GM):
                    src, col, _ = g_region(l, m)
                    nc.scalar.activation(
                        outs[m][:, (m % 4) * 32:(m % 4 + 1) * 32],
                        src[:, col:col + 32],
                        funcs[m],
                        bias=bs_sb[:, l * GM + m: l * GM + m + 1],
                    )
                t1 = sb_pool.tile([128, 128], f32, tag="t1")
                t2 = sb_pool.tile([128, 128], f32, tag="t2")
                tcn = sb_pool.tile([128, 128], f32, tag="tcn")
                nc.vector.tensor_mul(t1[:], s_f[:], c_old[:])
                nc.vector.tensor_mul(t2[:], s_i[:], tg[:])
                nc.vector.tensor_add(c_new[:], t1[:], t2[:])
                nc.scalar.activation(tcn[:], c_new[:], AF.Tanh)
                nc.vector.tensor_mul(h_new[:], s_o[:], tcn[:])
                return h_new

            def emit_y(src_h, dst):
                for k in range(KT):
                    nc.tensor.matmul(
                        dst[:, 0:BC],
                        w2_t(k),
                        src_h[:, k * 32:(k + 1) * 32],
                        start=(k == 0),
                        stop=(k == KT - 1),
                    )

            def emit_yout(y_src, y_pair, s):
                """stored output row (off the critical path), on DVE."""
                if zero_bias:
                    nc.vector.tensor_scalar_mul(y_pair[:, s, :], y_src[:, 0:BC],
                                                1.0)
                else:
                    nc.vector.tensor_scalar_add(y_pair[:, s, :],
                                                y_src[:, 0:BC],
                                                b2_sb[:, 0:1])

            # prologue: h-part of layer-0 gates for step 0 (h=0, but also
            # initializes the PSUM accumulation groups for the first GIN0),
            # plus z for step 0: x-part and the ground-truth feedback term
            # (in-loop, step 0's W_fb @ h1(=0) adds nothing on top).
            emit_gh(0, h_sb[0][1], 0, GM)
            emit_z_xpart(0, zp_t)
            for m in range(KT):
                nc.tensor.matmul(
                    zp_t[:, m * 32:(m + 1) * 32],
                    w1_t(KT - 1, m),
                    x0fb_sb[:],
                    start=False,
                    stop=False,
                )
            # NOTE: x_a holds steps 0..UH-1 (prologue DMA); body 0's top DMA
            # fills x_b; the mid-body DMA refills x_a for the next iteration.

            n_iters = n_steps // U
            hint = (tuple(mybir.EngineType[e] for e in
                          ("PE", "Activation", "DVE", "SP"))
                    if HINTS else ())
            C = U * 128  # iter stride: U*KT*BC x-cols == U*OUT ys-rows
            for _rep in range(reps):
              with tc.For_i(0, n_iters * C, C,
                            staggered_reset=STAGGERED,
                            hint_engines=hint) as it:
                # it*4 == iteration_index * U * IN (xT_flat row stride)
                emit_xdma(x_ab[1], it * 4 + UH * IN)   # this iter, 2nd half
                y_pair = sb_pool.tile([128, U, BC], f32, tag="y_pair")
                for s in range(U):
                    h1_old = h_sb[1][1 - s % 2]
                    # PE emission order is chosen so that independent w_hh
                    # matmul chunks cover every cross-engine serial gap
                    # (z relu / cell0 / cell1).
                    emit_z_hfb(h1_old, zp_t)
                    emit_gh(1, h1_old, 0, 8)
                    z_bf = emit_zact(zp_t)
                    emit_gin(0, z_bf)
                    emit_gh(1, h1_old, 8, 16)
                    # pre-accumulate next step's z x-part (cell0 PE filler);
                    # final step of final iter writes a phantom epoch from
                    # the zero-padded prefetch -- opened but never read.
                    emit_z_xpart(s + 1, zp_t)
                    h0n = emit_cell(0, s % 2)
                    emit_gin(1, h0n)
                    # software pipeline: next step's layer-0 h-part
                    emit_gh(0, h0n, 0, 12)
                    h1n = emit_cell(1, s % 2)
                    emit_y(h1n, yp_t)
                    emit_gh(0, h0n, 12, 16)
                    emit_yout(yp_t, y_pair, s)
                    if s == UH - 1:
                        # x_a fully consumed (its last z-xpart, for step UH,
                        # was just pre-emitted) -> prefetch next iteration's
                        # first half.
                        emit_xdma(x_ab[0], it * 4 + U * IN)
                nc.sync.dma_start(
                    out=ys_flat[ds(it, U * 128), :].rearrange(
                        "(t o) b -> o t b", t=U),
                    in_=y_pair[:],
                )

    _coalesce_pe_updates(nc, group=int(os.environ.get("KCOAL", "4")))
    _split_waits(nc)
    return nc


def _coalesce_pe_updates(nc, group=4):
    """The PE posts a +1 semaphore update every ~25ns during matmul bursts --
    faster than the semaphore block drains them -- so consumers observe PE
    progress ~0.7-1us late.  Thin the updates: within each run of
    consecutive PE matmuls only every group-th matmul keeps its +1 post
    (walrus requires engine sem-inc values of exactly 1), and the whole
    count space is renumbered: every wait rounds up to the next surviving
    post, and the loop's +/-total reset arithmetic (add/sub-imm, which DO
    allow arbitrary values) is patched to the new body total.  Runs break
    at any matmul that itself carries a wait, so a deferred post can never
    sit behind a PE stall (no deadlock through another engine)."""
    if group <= 1:
        return
    import bisect
    blocks = nc.m.functions[0].blocks
    # the PE completion semaphore: the one matmuls post to
    sem_id = None
    body_idx, body_mm = 0, 0
    for bi, bb in enumerate(blocks):
        n = 0
        for inst in bb.instructions:
            if type(inst).__name__ == "InstMatmult" and inst.sync_info:
                for u in inst.sync_info.on_update or []:
                    if u.update_mode == "sem-inc" and u.update_value == 1:
                        sem_id = u.id if sem_id is None else sem_id
                        n += 1
        if n > body_mm:
            body_idx, body_mm = bi, n
    if sem_id is None or body_mm == 0:
        return
    base = 0             # posts before the body block (prologue total)
    for bb in blocks[:body_idx]:
        for inst in bb.instructions:
            if inst.sync_info:
                for u in inst.sync_info.on_update or []:
                    if u.id == sem_id and u.update_mode == "sem-inc":
                        base += u.update_value

    bb = blocks[body_idx]
    posted = []          # OLD cumulative body count at each surviving post
    cum = 0              # old cumulative body count
    run = []             # matmuls in current run posting +1 to sem_id

    def strip(inst):
        si = inst.sync_info
        inst.sync_info = mybir.SyncInfo(
            on_wait=si.on_wait,
            on_update=[u for u in si.on_update
                       if not (u.id == sem_id
                               and u.update_mode == "sem-inc")])

    def flush(run):
        nonlocal cum
        i = 0
        while i < len(run):
            g = run[i:i + group]
            for inst in g[:-1]:
                strip(inst)
            cum += len(g)
            posted.append(cum)   # g[-1] keeps its +1 post
            i += len(g)
        run.clear()

    for inst in bb.instructions:
        si = inst.sync_info
        upds = [u for u in (si.on_update if si else []) or []
                if u.id == sem_id]
        is_run_mm = (type(inst).__name__ == "InstMatmult"
                     and len(upds) == 1
                     and upds[0].update_mode == "sem-inc"
                     and upds[0].update_value == 1)
        has_wait = bool(si and si.on_wait)
        if is_run_mm and not has_wait:
            run.append(inst)
            continue
        if is_run_mm:           # waiting matmul: break, then start new run
            flush(run)
            run.append(inst)
            continue
        if upds:                 # non-matmul poster: flush, keep its post
            flush(run)
            for u in upds:
                if u.update_mode == "sem-inc":
                    cum += u.update_value
                    posted.append(cum)
        elif (run and getattr(inst, "engine", None) == mybir.EngineType.PE
                and has_wait):
            flush(run)           # a waiting PE instruction can stall the run
            # (plain LDWEIGHTS etc. interleave runs harmlessly)
    flush(run)
    old_total, new_total = cum, len(posted)

    for bb2 in blocks:
        for inst in bb2.instructions:
            si = inst.sync_info
            if not si:
                continue
            for w in si.on_wait or []:
                # waits above `base` live in the body count window
                if (w.id == sem_id and w.wait_mode == "sem-ge-imm"
                        and w.wait_value and w.wait_value > base):
                    j = bisect.bisect_left(posted, w.wait_value - base)
                    w.wait_value = base + min(j + 1, new_total)
            for u in si.on_update or []:
                # the loop skip/reset arithmetic references the body total
                if (u.id == sem_id and u.update_value == old_total
                        and u.update_mode in ("sem-add-imm", "sem-sub-imm")):
                    u.update_value = new_total


def _split_waits(nc, cap=1):
    """walrus encodes a single sync-wait command per instruction.  Hoist
    excess waits from any instruction onto inserted single-wait NOPs on
    the same engine — semantically identical, the engine just blocks on
    the NOPs first."""
    for bb in nc.m.functions[0].blocks:
        new_insts = []
        for inst in bb.instructions:
            if (inst.sync_info is not None
                    and len(inst.sync_info.on_wait or ()) > cap):
                waits = list(inst.sync_info.on_wait)
                head, tail = waits[:-cap], waits[-cap:]
                for w in head:
                    nop = mybir.InstNoOp(
                        name=nc.get_next_instruction_name(),
                        engine=inst.engine,
                        ins=[],
                        outs=[],
                        sync_info=mybir.SyncInfo(on_wait=[w], on_update=[]),
                    )
                    nc.register_instruction(nop)
                    new_insts.append(nop)
                inst.sync_info = mybir.SyncInfo(
                    on_wait=tail, on_update=inst.sync_info.on_update)
            new_insts.append(inst)
        bb.instructions[:] = new_insts


def _pack_inputs(x, w1, b1, w_ih, w_hh, b_ih, b_hh, w2, b2, n_steps=T):
    """Host-side packing shared by all cores (weights) + per-core x."""
    w1T = np.ascontiguousarray(w1.T).astype(BF16)
    w_fb = (w1[:, IN - OUT:].astype(np.float32)
            @ w2.astype(np.float32))            # [H, H]
    wfbT = np.ascontiguousarray(w_fb.T).astype(BF16)
    wihT = np.ascontiguousarray(w_ih.transpose(0, 2, 1)).astype(BF16)
    whhT = np.ascontiguousarray(w_hh.transpose(0, 2, 1)).astype(BF16)
    w2T = np.ascontiguousarray(w2.T).astype(BF16)
    b1_eff = (b1.astype(np.float32)
              + w1[:, IN - OUT:].astype(np.float32) @ b2.astype(np.float32))
    b1p = np.ascontiguousarray(b1_eff.reshape(KT, 128).T).astype(np.float32)
    bsum = (b_ih + b_hh).astype(np.float32)
    bsp = np.ascontiguousarray(bsum.reshape(L, GM, 128).transpose(0, 2, 1))
    b2p = np.ascontiguousarray(b2.reshape(1, 128).T).astype(np.float32)
    shared = dict(w1T=w1T, wfbT=wfbT, wihT=wihT, whhT=whhT, w2T=w2T,
                  b1p=b1p, bsp=bsp, b2p=b2p)
    in_maps = []
    for c in range(NCORES):
        xs = x[:n_steps, c * BC:(c + 1) * BC, :]
        xT = np.zeros((n_steps + U, IN, BC), BF16)
        xT[:n_steps] = xs.transpose(0, 2, 1).astype(BF16)
        in_maps.append(dict(xT=xT, **shared))
    zero_bias = (not b1_eff.any()) and (not bsum.any()) and (not b2.any())
    return in_maps, zero_bias


def kernel(x, w1, b1, w_ih, w_hh, b_ih, b_hh, w2, b2):
    x = np.asarray(x, dtype=np.float32)
    args = [np.asarray(a, dtype=np.float32) for a in
            (w1, b1, w_ih, w_hh, b_ih, b_hh, w2, b2)]
    in_maps, zero_bias = _pack_inputs(x, *args)
    nc = build_program(zero_bias)
    res = run_bass_kernel_spmd(nc, in_maps, list(range(NCORES)))
    outs = [np.asarray(r["ysT"]).transpose(0, 2, 1) for r in res.results]
    return np.concatenate(outs, axis=1).astype(np.float32)
